# revision 28
# baseline (speedup 1.0000x reference)
"""Multi-head attention (B=32, S=512, D=768, H=12, E=64) on 8 Trainium2 cores.

Sharding: data-parallel over batch — each of the 8 cores processes 4 batches
with a full copy of the weights. No collectives.

v2 design notes. Every engine queue is in-order, so emission order IS the
schedule; the kernel is laid out so the PE stream never waits:

  - Weights DMA'd tensor-by-tensor (Wq first) so Q-proj starts ~7us in
    instead of waiting ~26us for all weights.
  - Softmax denominator: pt tiles pair-summed (one Pool add + one DVE add,
    bf16), r = two accumulating K=128 ones-matmuls (1 cyc/row), 1/r via
    reciprocal_approx_fast (DVE custom op, ~18 bits), then broadcast to all
    128 partitions by a DRAM-bounce DMA with a stride-0 partition read and
    bf16 cast in flight. The normalize multiply is fused into the AV
    PSUM->SBUF eviction (one DVE mul per head) — no ScalarE copies.
    (gpsimd library ucode — partition_all_reduce etc. — is NOT available
    in this image (BEDROCK=1) and crashes the device; DVE TensorTensor
    divide is rejected by the ISA checker. Hence this formulation.)
  - exp processes two key-blocks per instruction ([128,1024] PSUM tiles) to
    amortize ScalarE access-latency overhead.
  - The NEXT batch's transposes + QKV projections AND the PREVIOUS batch's
    out-projection are emitted as work units interleaved between attention
    heads of the current batch, filling the PE bubbles left by the
    scores->exp->AV dependency chain. Head pairs share one av PSUM tile
    (disjoint partition halves) to double the eviction lookahead.

Per-batch PE work: transposes 3072cy + QK 36864 + V 18432 + scores 24576 +
AV 24576 + r 10240 + out-proj 18432 ~= 136k cycles ~= 57us @2.4GHz;
everything else (ScalarE exp, DVE sums/recip/evictions, Pool sums/SWDGE)
fits underneath. Measured: ~350us vs 547us baseline (~1.55x), rel_err
5.26e-3 (unchanged from baseline).
"""

import numpy as np

import concourse.bass as bass
import concourse.tile as tile
import concourse.mybir as mybir
from concourse import bacc
from concourse import bass_utils
from concourse.masks import make_identity

B, S, D, H, E = 32, 512, 768, 12, 64
NCORES = 8
BL = B // NCORES          # local batches per core
CD = D // 128             # 6 chunks of 128 over d / he
F32 = mybir.dt.float32
BF16 = mybir.dt.bfloat16
AF = mybir.ActivationFunctionType


def build_nc():
    nc = bacc.Bacc(None)

    x_d = nc.dram_tensor("x", [BL, S, D], F32, kind="ExternalInput")
    wq_d = nc.dram_tensor("Wq", [H, D, E], F32, kind="ExternalInput")
    wk_d = nc.dram_tensor("Wk", [H, D, E], F32, kind="ExternalInput")
    wv_d = nc.dram_tensor("Wv", [H, D, E], F32, kind="ExternalInput")
    bq_d = nc.dram_tensor("bq", [H, E], F32, kind="ExternalInput")
    bk_d = nc.dram_tensor("bk", [H, E], F32, kind="ExternalInput")
    bv_d = nc.dram_tensor("bv", [H, E], F32, kind="ExternalInput")
    wo_d = nc.dram_tensor("Wo", [D, D], F32, kind="ExternalInput")
    bo_d = nc.dram_tensor("bo", [D], F32, kind="ExternalInput")
    out_d = nc.dram_tensor("out", [BL, S, D], F32, kind="ExternalOutput")
    # DRAM bounce buffer for the 1/r partition-broadcast (one slot per
    # (batch, head) so slots are never reused -> no WAR hazards)
    rscr_d = nc.dram_tensor("rscr", [BL * H, S], F32, kind="Internal")

    with nc.allow_low_precision(reason="bf16 intermediates"), \
         tile.TileContext(nc) as tc:
        with (
            tc.tile_pool(name="singles", bufs=1) as singles,
            tc.tile_pool(name="wstage", bufs=6) as wstage_pool,
            tc.tile_pool(name="xnat", bufs=3) as xnat_pool,
            tc.tile_pool(name="xt", bufs=2) as xt_pool,
            tc.tile_pool(name="qk", bufs=2) as qk_pool,
            tc.tile_pool(name="vv", bufs=2) as v_pool,
            tc.tile_pool(name="ot", bufs=2) as ot_pool,
            tc.tile_pool(name="pt", bufs=6) as pt_pool,
            tc.tile_pool(name="psm", bufs=4) as psum_pool,
            tc.tile_pool(name="rbc", bufs=2) as rbc_pool,
            tc.tile_pool(name="ostage", bufs=2) as out_pool,
            # PSUM budget (16KB/partition): pps 2x2KB + sc 2x4KB + av 2x2KB
            tc.tile_pool(name="pps", bufs=2, space="PSUM") as pps,
            tc.tile_pool(name="sc_ps", bufs=2, space="PSUM") as sc_ps,
            tc.tile_pool(name="av_ps", bufs=2, space="PSUM") as av_ps,
        ):
            # ---- constants ----
            ident = singles.tile([128, 128], BF16, tag="ident")
            make_identity(nc, ident)
            ones_col = singles.tile([128, 1], BF16, tag="onesc")
            nc.vector.memset(ones_col, 1.0)

            # ---- x(0) first (SWDGE f32->bf16 cast), then bias broadcasts
            # on the same queue ----
            xn_tiles = {}

            def emit_x_load(b):
                tl = []
                for t4 in range(4):
                    xn = xnat_pool.tile([128, D], BF16)
                    nc.gpsimd.dma_start(
                        out=xn, in_=x_d.ap()[b, t4 * 128:(t4 + 1) * 128, :]
                    )
                    tl.append(xn)
                xn_tiles[b] = tl

            emit_x_load(0)

            bv_bc = singles.tile([128, D], F32, tag="bvbc")
            bo_bc = singles.tile([128, D], F32, tag="bobc")
            for dst, src_d in ((bv_bc, bv_d), (bo_bc, bo_d)):
                f = src_d.ap().flatten()
                nc.gpsimd.dma_start(
                    out=dst,
                    in_=bass.AP(tensor=f.tensor, offset=f.offset,
                                ap=[[0, 128]] + [list(p) for p in f.ap]),
                )

            # ---- weights tensor-by-tensor; Wq + QK biases first so Q-proj
            # can start as soon as Wq lands ----
            w_sb = {}
            bq_sb = singles.tile([128, CD], F32, tag="bq")
            bk_sb = singles.tile([128, CD], F32, tag="bk")

            def load_w(name, wd, cast_fn):
                t = singles.tile([128, CD, D], BF16, tag=f"w{name}")
                src = wd.ap().rearrange("h (c p) e -> c p h e", p=128)
                for c in range(CD):
                    stg = wstage_pool.tile([128, D], F32)
                    nc.sync.dma_start(
                        out=stg.rearrange("p (h e) -> p h e", e=E), in_=src[c]
                    )
                    cast_fn(t[:, c, :], stg)
                w_sb[name] = t

            load_w("q", wq_d, lambda o, i: nc.vector.tensor_copy(out=o, in_=i))
            nc.sync.dma_start(
                out=bq_sb, in_=bq_d.ap().flatten().rearrange("(m p) -> p m", p=128)
            )
            nc.sync.dma_start(
                out=bk_sb, in_=bk_d.ap().flatten().rearrange("(m p) -> p m", p=128)
            )
            load_w("k", wk_d, lambda o, i: nc.scalar.copy(out=o, in_=i))
            load_w("v", wv_d, lambda o, i: nc.gpsimd.tensor_copy(out=o, in_=i))
            wo_sb = singles.tile([128, CD, D], BF16, tag="wo")
            wo_src = wo_d.ap().rearrange("(c p) n -> c p n", p=128)
            for c in range(CD):
                stg = wstage_pool.tile([128, D], F32)
                nc.sync.dma_start(out=stg, in_=wo_src[c])
                nc.vector.tensor_copy(out=wo_sb[:, c, :], in_=stg)

            xt_tiles = {}
            qk_tiles = {}
            v_tiles = {}
            ot_tiles = {}

            def p1_units(b):
                """The next batch's prep as a list of PE work units, to be
                interleaved between attention heads of the current batch."""
                units = []
                xt = xt_pool.tile([128, CD, S], BF16)
                xt_tiles[b] = xt

                def transpose_unit(t4):
                    # one XBAR DMA transposes the whole [128 tok, 768 d]
                    # block into xt[:, :, t4] — no PE/DVE involvement
                    def emit():
                        nc.sync.dma_start_transpose(
                            out=xt[:, :, t4 * 128:(t4 + 1) * 128],
                            in_=xn_tiles[b][t4],
                        )
                    return emit

                for t4 in range(4):
                    units.append(transpose_unit(t4))

                v_units = []
                v_sb = v_pool.tile([128, 4, D], BF16)
                v_tiles[b] = v_sb

                def v_unit(t4, n):
                    def emit():
                        ps = pps.tile([128, S], F32, tag="ps")
                        for c in range(CD):
                            nc.tensor.matmul(
                                ps[:, 0:384],
                                lhsT=xt[:, c, t4 * 128:(t4 + 1) * 128],
                                rhs=w_sb["v"][:, c, n * 384:(n + 1) * 384],
                                start=(c == 0),
                                stop=(c == CD - 1),
                            )
                        nc.vector.tensor_add(
                            out=v_sb[:, t4, n * 384:(n + 1) * 384],
                            in0=ps[:, 0:384],
                            in1=bv_bc[:, n * 384:(n + 1) * 384],
                        )
                    return emit

                for t4 in range(4):
                    for n in range(2):
                        v_units.append(v_unit(t4, n))

                qT = qk_pool.tile([128, CD, S], BF16, tag="qT")
                kT = qk_pool.tile([128, CD, S], BF16, tag="kT")
                qk_tiles[b] = (qT, kT)

                def qk_unit(dst, wname, bsb, m):
                    def emit():
                        ps = pps.tile([128, S], F32, tag="ps")
                        for c in range(CD):
                            nc.tensor.matmul(
                                ps,
                                lhsT=w_sb[wname][:, c, m * 128:(m + 1) * 128],
                                rhs=xt_tiles[b][:, c, :],
                                start=(c == 0),
                                stop=(c == CD - 1),
                            )
                        nc.scalar.add(
                            out=dst[:, m, :], in_=ps, add=bsb[:, m:m + 1],
                        )
                    return emit

                qk_units = [qk_unit(qT, "q", bq_sb, m) for m in range(CD)]
                qk_units += [qk_unit(kT, "k", bk_sb, m) for m in range(CD)]
                if b == 0:
                    # prologue: Wq lands first, Wk second, Wv third — order
                    # the GEMMs to chase the weight DMAs
                    units += qk_units + v_units
                else:
                    # steady state: V(t4) only needs its own t4 transposed;
                    # QK needs the full xt
                    units += v_units + qk_units
                return units

            def emit_attention(b, interleave):
                """Per head h: scores (2x [128,1024] psum) -> exp (ScalarE,
                two-block tiles) -> pair-sums (Pool+DVE, bf16) -> r via two
                accumulating ones-matmuls -> reciprocal_approx_fast -> 1/r
                broadcast to 128 partitions via a DRAM-bounce DMA (bf16 cast
                in flight) -> AV -> fused normalize-evict (DVE mul). Scores
                run 2 heads ahead; `interleave` units are popped between
                heads to fill PE bubbles."""
                qT, kT = qk_tiles[b]
                v_sb = v_tiles[b]
                oT = ot_pool.tile([128, CD, S], BF16, tag="oT")
                ot_tiles[b] = oT

                pt_tiles = {}
                sum_tiles = {}
                bc_tiles = {}

                def emit_scores(h):
                    hm, half = h // 2, 64 * (h % 2)
                    pts = []
                    for j in (0, 1):
                        sc = sc_ps.tile([128, 2, S], F32, tag="sc")
                        for i in (0, 1):
                            t4 = 2 * j + i
                            t4s = slice(t4 * 128, (t4 + 1) * 128)
                            nc.tensor.matmul(
                                sc[:, i, :],
                                lhsT=kT[half:half + 64, hm, t4s],
                                rhs=qT[half:half + 64, hm, :],
                                start=True, stop=True,
                            )
                        pt = pt_pool.tile([128, 2, S], BF16)
                        nc.scalar.activation(
                            out=pt.rearrange("p a b -> p (a b)"),
                            in_=sc.rearrange("p a b -> p (a b)"),
                            func=AF.Exp, scale=0.125)
                        pts.append(pt)
                    pt_tiles[h] = pts
                    s01 = psum_pool.tile([128, S], BF16, tag="s16")
                    nc.vector.tensor_add(out=s01, in0=pts[0][:, 0, :],
                                         in1=pts[0][:, 1, :])
                    s23 = psum_pool.tile([128, S], BF16, tag="s16")
                    nc.vector.tensor_add(out=s23, in0=pts[1][:, 0, :],
                                         in1=pts[1][:, 1, :])
                    sum_tiles[h] = (s01, s23)

                def emit_r(h):
                    """r + 1/r + broadcast, emitted one head ahead of AV so
                    the DMA bounce latency is covered by head h-1's AV."""
                    rp = pps.tile([128, S], F32, tag="ps")
                    for j, s in enumerate(sum_tiles[h]):
                        nc.tensor.matmul(
                            rp[0:1, :], lhsT=ones_col, rhs=s,
                            start=(j == 0), stop=(j == 1),
                        )
                    rinv = rbc_pool.tile([1, S], F32, tag="rinv")
                    nc.vector.reciprocal_approx_fast(out=rinv, in_=rp[0:1, :])
                    slot = rscr_d.ap()[b * H + h]
                    dout = nc.sync.dma_start(out=slot, in_=rinv)
                    bc = rbc_pool.tile([128, S], BF16, tag="bc")
                    f = slot.flatten()
                    din = nc.gpsimd.dma_start(
                        out=bc,
                        in_=bass.AP(tensor=f.tensor, offset=f.offset,
                                    ap=[[0, 128]] + [list(p) for p in f.ap]),
                    )
                    tile.add_dep_helper(din.ins, dout.ins, sync=True,
                                        reason="rinv bounce RAW")
                    bc_tiles[h] = bc
                    del sum_tiles[h]

                av_tiles = {}

                def emit_av(h):
                    hm, half = h // 2, 64 * (h % 2)
                    # head pairs share one av PSUM tile (disjoint partition
                    # halves) so evictions have twice the lookahead room
                    if h % 2 == 0:
                        av = av_ps.tile([128, S], F32, tag="av", name="av")
                        av_tiles[hm] = av
                    av = av_tiles[hm]
                    for t4 in range(4):
                        nc.tensor.matmul(
                            av[half:half + 64, :],
                            lhsT=v_sb[:, t4, h * 64:(h + 1) * 64],
                            rhs=pt_tiles[h][t4 // 2][:, t4 % 2, :],
                            start=(t4 == 0), stop=(t4 == 3),
                        )
                    # fused normalize + PSUM->SBUF eviction
                    nc.vector.tensor_mul(
                        out=oT[half:half + 64, hm, :],
                        in0=av[half:half + 64, :],
                        in1=bc_tiles[h][half:half + 64, :],
                    )
                    del pt_tiles[h], bc_tiles[h]

                emit_scores(0)
                emit_scores(1)
                emit_r(0)
                for h in range(H):
                    if h + 2 < H:
                        emit_scores(h + 2)
                    if h + 1 < H:
                        emit_r(h + 1)
                    emit_av(h)
                    for _ in range(2):
                        if interleave:
                            interleave.pop(0)()

            def p3_units(b):
                """Out-projection as 4 per-token-block units, interleaved
                into the NEXT batch's attention for PE cover."""
                def t4_unit(t4):
                    def emit():
                        oT = ot_tiles[b]
                        ostage = out_pool.tile([128, D], F32)
                        for n in range(2):
                            ps = pps.tile([128, S], F32, tag="ps")
                            for m in range(CD):
                                nc.tensor.matmul(
                                    ps[:, 0:384],
                                    lhsT=oT[:, m, t4 * 128:(t4 + 1) * 128],
                                    rhs=wo_sb[:, m, n * 384:(n + 1) * 384],
                                    start=(m == 0),
                                    stop=(m == CD - 1),
                                )
                            nc.vector.tensor_add(
                                out=ostage[:, n * 384:(n + 1) * 384],
                                in0=ps[:, 0:384],
                                in1=bo_bc[:, n * 384:(n + 1) * 384],
                            )
                        nc.sync.dma_start(
                            out=out_d.ap()[b, t4 * 128:(t4 + 1) * 128, :],
                            in_=ostage,
                        )
                    return emit
                return [t4_unit(t4) for t4 in range(4)]

            # ---- pipeline ----
            for unit in p1_units(0):
                unit()
            prev_p3 = []
            for b in range(BL):
                if b + 1 < BL:
                    emit_x_load(b + 1)
                    units = prev_p3 + p1_units(b + 1)
                else:
                    units = prev_p3
                emit_attention(b, units)
                for unit in units:
                    unit()
                prev_p3 = p3_units(b)
            for unit in prev_p3:
                unit()

    nc.finalize()
    return nc


_NC_CACHE = None


def _get_nc():
    global _NC_CACHE
    if _NC_CACHE is None:
        _NC_CACHE = build_nc()
    return _NC_CACHE


def run_spmd(inputs, trace=False, trace_cores=None):
    nc = _get_nc()
    x = np.ascontiguousarray(inputs["x"], dtype=np.float32)
    shared = {
        k: np.ascontiguousarray(inputs[k], dtype=np.float32)
        for k in ("Wq", "Wk", "Wv", "bq", "bk", "bv", "Wo", "bo")
    }
    in_maps = []
    for core in range(NCORES):
        m = dict(shared)
        m["x"] = np.ascontiguousarray(x[core * BL:(core + 1) * BL])
        in_maps.append(m)
    res = bass_utils.run_bass_kernel_spmd(
        nc, in_maps, core_ids=list(range(NCORES)),
        trace=trace, trace_cores=trace_cores,
    )
    return res


def kernel(**inputs) -> np.ndarray:
    res = run_spmd(inputs, trace=False)
    out = np.concatenate([res.results[i]["out"] for i in range(NCORES)], axis=0)
    return out.astype(np.float32)


# revision 31
# speedup vs baseline: 1.1879x; 1.1879x over previous
"""Multi-head attention (B=32, S=512, D=768, H=12, E=64) on 8 Trainium2 cores.

Sharding: data-parallel over batch — each of the 8 cores processes 4 batches
with a full copy of the weights. No collectives.

v2 design notes. Every engine queue is in-order, so emission order IS the
schedule; the kernel is laid out so the PE stream never waits:

  - Weights DMA'd tensor-by-tensor (Wq first) so Q-proj starts ~7us in
    instead of waiting ~26us for all weights.
  - Softmax denominator: pt tiles pair-summed (one Pool add + one DVE add,
    bf16), r = two accumulating K=128 ones-matmuls (1 cyc/row), 1/r via
    reciprocal_approx_fast (DVE custom op, ~18 bits), then broadcast to all
    128 partitions by a DRAM-bounce DMA with a stride-0 partition read and
    bf16 cast in flight. The normalize multiply is fused into the AV
    PSUM->SBUF eviction (one DVE mul per head) — no ScalarE copies.
    (gpsimd library ucode — partition_all_reduce etc. — is NOT available
    in this image (BEDROCK=1) and crashes the device; DVE TensorTensor
    divide is rejected by the ISA checker. Hence this formulation.)
  - exp processes two key-blocks per instruction ([128,1024] PSUM tiles) to
    amortize ScalarE access-latency overhead.
  - The NEXT batch's transposes + QKV projections AND the PREVIOUS batch's
    out-projection are emitted as work units interleaved between attention
    heads of the current batch, filling the PE bubbles left by the
    scores->exp->AV dependency chain. Head pairs share one av PSUM tile
    (disjoint partition halves) to double the eviction lookahead.

Per-batch PE work: transposes 3072cy + QK 36864 + V 18432 + scores 24576 +
AV 24576 + r 10240 + out-proj 18432 ~= 136k cycles ~= 57us @2.4GHz;
everything else (ScalarE exp, DVE sums/recip/evictions, Pool sums/SWDGE)
fits underneath. Measured: ~350us vs 547us baseline (~1.55x), rel_err
5.26e-3 (unchanged from baseline).
"""

import numpy as np

import concourse.bass as bass
import concourse.tile as tile
import concourse.mybir as mybir
from concourse import bacc
from concourse import bass_utils
from concourse.masks import make_identity

B, S, D, H, E = 32, 512, 768, 12, 64
NCORES = 8
BL = B // NCORES          # local batches per core
CD = D // 128             # 6 chunks of 128 over d / he
F32 = mybir.dt.float32
BF16 = mybir.dt.bfloat16
AF = mybir.ActivationFunctionType


def build_nc():
    nc = bacc.Bacc(None)

    x_d = nc.dram_tensor("x", [BL, S, D], F32, kind="ExternalInput")
    wq_d = nc.dram_tensor("Wq", [H, D, E], F32, kind="ExternalInput")
    wk_d = nc.dram_tensor("Wk", [H, D, E], F32, kind="ExternalInput")
    wv_d = nc.dram_tensor("Wv", [H, D, E], F32, kind="ExternalInput")
    bq_d = nc.dram_tensor("bq", [H, E], F32, kind="ExternalInput")
    bk_d = nc.dram_tensor("bk", [H, E], F32, kind="ExternalInput")
    bv_d = nc.dram_tensor("bv", [H, E], F32, kind="ExternalInput")
    wo_d = nc.dram_tensor("Wo", [D, D], F32, kind="ExternalInput")
    bo_d = nc.dram_tensor("bo", [D], F32, kind="ExternalInput")
    out_d = nc.dram_tensor("out", [BL, S, D], F32, kind="ExternalOutput")
    # DRAM bounce buffer for the 1/r partition-broadcast (one slot per
    # (batch, head) so slots are never reused -> no WAR hazards)
    rscr_d = nc.dram_tensor("rscr", [BL * H, S], F32, kind="Internal")

    with nc.allow_low_precision(reason="bf16 intermediates"), \
         tile.TileContext(nc) as tc:
        with (
            tc.tile_pool(name="singles", bufs=1) as singles,
            tc.tile_pool(name="wstage", bufs=6) as wstage_pool,
            tc.tile_pool(name="xnat", bufs=3) as xnat_pool,
            tc.tile_pool(name="xt", bufs=2) as xt_pool,
            tc.tile_pool(name="qk", bufs=2) as qk_pool,
            tc.tile_pool(name="vv", bufs=2) as v_pool,
            tc.tile_pool(name="ot", bufs=2) as ot_pool,
            tc.tile_pool(name="pt", bufs=6) as pt_pool,
            tc.tile_pool(name="psm", bufs=4) as psum_pool,
            tc.tile_pool(name="rbc", bufs=2) as rbc_pool,
            tc.tile_pool(name="ostage", bufs=2) as out_pool,
            # PSUM budget (16KB/partition): pps 2x2KB + sc 2x4KB + av 2x2KB
            tc.tile_pool(name="pps", bufs=2, space="PSUM") as pps,
            tc.tile_pool(name="sc_ps", bufs=2, space="PSUM") as sc_ps,
            tc.tile_pool(name="av_ps", bufs=2, space="PSUM") as av_ps,
        ):
            # ---- constants ----
            ident = singles.tile([128, 128], BF16, tag="ident")
            make_identity(nc, ident)
            ones_col = singles.tile([128, 1], BF16, tag="onesc")
            nc.vector.memset(ones_col, 1.0)

            # ---- x(0) first (SWDGE f32->bf16 cast), then bias broadcasts
            # on the same queue ----
            xn_tiles = {}

            def emit_x_load(b):
                tl = []
                for t4 in range(4):
                    xn = xnat_pool.tile([128, D], BF16)
                    nc.gpsimd.dma_start(
                        out=xn, in_=x_d.ap()[b, t4 * 128:(t4 + 1) * 128, :]
                    )
                    tl.append(xn)
                xn_tiles[b] = tl

            emit_x_load(0)

            bv_bc = singles.tile([128, D], F32, tag="bvbc")
            bo_bc = singles.tile([128, D], F32, tag="bobc")
            for dst, src_d in ((bv_bc, bv_d), (bo_bc, bo_d)):
                f = src_d.ap().flatten()
                nc.gpsimd.dma_start(
                    out=dst,
                    in_=bass.AP(tensor=f.tensor, offset=f.offset,
                                ap=[[0, 128]] + [list(p) for p in f.ap]),
                )

            # ---- weights tensor-by-tensor; Wq + QK biases first so Q-proj
            # can start as soon as Wq lands ----
            w_sb = {}
            bq_sb = singles.tile([128, CD], F32, tag="bq")
            bk_sb = singles.tile([128, CD], F32, tag="bk")

            def load_w(name, wd, cast_fn):
                t = singles.tile([128, CD, D], BF16, tag=f"w{name}")
                src = wd.ap().rearrange("h (c p) e -> c p h e", p=128)
                for c in range(CD):
                    stg = wstage_pool.tile([128, D], F32)
                    nc.sync.dma_start(
                        out=stg.rearrange("p (h e) -> p h e", e=E), in_=src[c]
                    )
                    cast_fn(t[:, c, :], stg)
                w_sb[name] = t

            load_w("q", wq_d, lambda o, i: nc.vector.tensor_copy(out=o, in_=i))
            nc.sync.dma_start(
                out=bq_sb, in_=bq_d.ap().flatten().rearrange("(m p) -> p m", p=128)
            )
            nc.sync.dma_start(
                out=bk_sb, in_=bk_d.ap().flatten().rearrange("(m p) -> p m", p=128)
            )
            load_w("k", wk_d, lambda o, i: nc.scalar.copy(out=o, in_=i))
            load_w("v", wv_d, lambda o, i: nc.gpsimd.tensor_copy(out=o, in_=i))
            wo_sb = singles.tile([128, CD, D], BF16, tag="wo")
            wo_src = wo_d.ap().rearrange("(c p) n -> c p n", p=128)
            for c in range(CD):
                stg = wstage_pool.tile([128, D], F32)
                nc.sync.dma_start(out=stg, in_=wo_src[c])
                nc.vector.tensor_copy(out=wo_sb[:, c, :], in_=stg)

            xt_tiles = {}
            qk_tiles = {}
            v_tiles = {}
            ot_tiles = {}

            def p1_units(b):
                """The next batch's prep as a list of PE work units, to be
                interleaved between attention heads of the current batch."""
                units = []
                xt = xt_pool.tile([128, CD, S], BF16)
                xt_tiles[b] = xt

                def transpose_unit(t4, cg, ncg):
                    def emit():
                        xn = xn_tiles[b][t4]
                        tp = pps.tile([128, S], BF16, tag="ps")
                        for j in range(ncg):
                            c = cg + j
                            nc.tensor.transpose(
                                tp[:, j * 128:(j + 1) * 128],
                                xn[:, c * 128:(c + 1) * 128],
                                ident,
                            )
                        nc.vector.tensor_copy(
                            out=xt[:, cg:cg + ncg, t4 * 128:(t4 + 1) * 128],
                            in_=tp[:, 0:ncg * 128].rearrange(
                                "p (c q) -> p c q", q=128),
                        )
                    return emit

                for t4 in range(4):
                    for cg, ncg in ((0, 4), (4, 2)):
                        units.append(transpose_unit(t4, cg, ncg))

                v_units = []
                v_sb = v_pool.tile([128, 4, D], BF16)
                v_tiles[b] = v_sb

                def v_unit(t4, n):
                    def emit():
                        ps = pps.tile([128, S], F32, tag="ps")
                        for c in range(CD):
                            nc.tensor.matmul(
                                ps[:, 0:384],
                                lhsT=xt[:, c, t4 * 128:(t4 + 1) * 128],
                                rhs=w_sb["v"][:, c, n * 384:(n + 1) * 384],
                                start=(c == 0),
                                stop=(c == CD - 1),
                            )
                        nc.vector.tensor_add(
                            out=v_sb[:, t4, n * 384:(n + 1) * 384],
                            in0=ps[:, 0:384],
                            in1=bv_bc[:, n * 384:(n + 1) * 384],
                        )
                    return emit

                for t4 in range(4):
                    for n in range(2):
                        v_units.append(v_unit(t4, n))

                qT = qk_pool.tile([128, CD, S], BF16, tag="qT")
                kT = qk_pool.tile([128, CD, S], BF16, tag="kT")
                qk_tiles[b] = (qT, kT)

                def qk_unit(dst, wname, bsb, m):
                    def emit():
                        ps = pps.tile([128, S], F32, tag="ps")
                        for c in range(CD):
                            nc.tensor.matmul(
                                ps,
                                lhsT=w_sb[wname][:, c, m * 128:(m + 1) * 128],
                                rhs=xt_tiles[b][:, c, :],
                                start=(c == 0),
                                stop=(c == CD - 1),
                            )
                        nc.scalar.add(
                            out=dst[:, m, :], in_=ps, add=bsb[:, m:m + 1],
                        )
                    return emit

                qk_units = [qk_unit(qT, "q", bq_sb, m) for m in range(CD)]
                qk_units += [qk_unit(kT, "k", bk_sb, m) for m in range(CD)]
                if b == 0:
                    # prologue: Wq lands first, Wk second, Wv third — order
                    # the GEMMs to chase the weight DMAs
                    units += qk_units + v_units
                else:
                    # steady state: V(t4) only needs its own t4 transposed;
                    # QK needs the full xt
                    units += v_units + qk_units
                return units

            def emit_attention(b, interleave):
                """Per head h: scores (2x [128,1024] psum) -> exp (ScalarE,
                two-block tiles) -> pair-sums (Pool+DVE, bf16) -> r via two
                accumulating ones-matmuls -> reciprocal_approx_fast -> 1/r
                broadcast to 128 partitions via a DRAM-bounce DMA (bf16 cast
                in flight) -> AV -> fused normalize-evict (DVE mul). Scores
                run 2 heads ahead; `interleave` units are popped between
                heads to fill PE bubbles."""
                qT, kT = qk_tiles[b]
                v_sb = v_tiles[b]
                oT = ot_pool.tile([128, CD, S], BF16, tag="oT")
                ot_tiles[b] = oT

                pt_tiles = {}
                sum_tiles = {}
                bc_tiles = {}

                def emit_scores(h):
                    hm, half = h // 2, 64 * (h % 2)
                    pts = []
                    for j in (0, 1):
                        sc = sc_ps.tile([128, 2, S], F32, tag="sc")
                        for i in (0, 1):
                            t4 = 2 * j + i
                            t4s = slice(t4 * 128, (t4 + 1) * 128)
                            nc.tensor.matmul(
                                sc[:, i, :],
                                lhsT=kT[half:half + 64, hm, t4s],
                                rhs=qT[half:half + 64, hm, :],
                                start=True, stop=True,
                            )
                        pt = pt_pool.tile([128, 2, S], BF16)
                        nc.scalar.activation(
                            out=pt.rearrange("p a b -> p (a b)"),
                            in_=sc.rearrange("p a b -> p (a b)"),
                            func=AF.Exp, scale=0.125)
                        pts.append(pt)
                    pt_tiles[h] = pts
                    s01 = psum_pool.tile([128, S], BF16, tag="s16")
                    nc.vector.tensor_add(out=s01, in0=pts[0][:, 0, :],
                                         in1=pts[0][:, 1, :])
                    s23 = psum_pool.tile([128, S], BF16, tag="s16")
                    nc.vector.tensor_add(out=s23, in0=pts[1][:, 0, :],
                                         in1=pts[1][:, 1, :])
                    sum_tiles[h] = (s01, s23)

                def emit_r(h):
                    """r + 1/r + broadcast, emitted one head ahead of AV so
                    the DMA bounce latency is covered by head h-1's AV."""
                    rp = pps.tile([128, S], F32, tag="ps")
                    for j, s in enumerate(sum_tiles[h]):
                        nc.tensor.matmul(
                            rp[0:1, :], lhsT=ones_col, rhs=s,
                            start=(j == 0), stop=(j == 1),
                        )
                    rinv = rbc_pool.tile([1, S], F32, tag="rinv")
                    nc.vector.reciprocal_approx_fast(out=rinv, in_=rp[0:1, :])
                    slot = rscr_d.ap()[b * H + h]
                    dout = nc.sync.dma_start(out=slot, in_=rinv)
                    bc = rbc_pool.tile([128, S], F32, tag="bc")
                    f = slot.flatten()
                    din = nc.sync.dma_start(
                        out=bc,
                        in_=bass.AP(tensor=f.tensor, offset=f.offset,
                                    ap=[[0, 128]] + [list(p) for p in f.ap]),
                    )
                    tile.add_dep_helper(din.ins, dout.ins, sync=True,
                                        reason="rinv bounce RAW")
                    bc_tiles[h] = bc
                    del sum_tiles[h]

                av_tiles = {}

                def emit_av(h):
                    hm, half = h // 2, 64 * (h % 2)
                    # head pairs share one av PSUM tile (disjoint partition
                    # halves) so evictions have twice the lookahead room
                    if h % 2 == 0:
                        av = av_ps.tile([128, S], F32, tag="av", name="av")
                        av_tiles[hm] = av
                    av = av_tiles[hm]
                    for t4 in range(4):
                        nc.tensor.matmul(
                            av[half:half + 64, :],
                            lhsT=v_sb[:, t4, h * 64:(h + 1) * 64],
                            rhs=pt_tiles[h][t4 // 2][:, t4 % 2, :],
                            start=(t4 == 0), stop=(t4 == 3),
                        )
                    # fused normalize + PSUM->SBUF eviction
                    nc.vector.tensor_mul(
                        out=oT[half:half + 64, hm, :],
                        in0=av[half:half + 64, :],
                        in1=bc_tiles[h][half:half + 64, :],
                    )
                    del pt_tiles[h], bc_tiles[h]

                emit_scores(0)
                emit_scores(1)
                emit_r(0)
                for h in range(H):
                    if h + 2 < H:
                        emit_scores(h + 2)
                    if h + 1 < H:
                        emit_r(h + 1)
                    emit_av(h)
                    for _ in range(2):
                        if interleave:
                            interleave.pop(0)()

            def p3_units(b):
                """Out-projection as 4 per-token-block units, interleaved
                into the NEXT batch's attention for PE cover."""
                def t4_unit(t4):
                    def emit():
                        oT = ot_tiles[b]
                        ostage = out_pool.tile([128, D], F32)
                        for n in range(2):
                            ps = pps.tile([128, S], F32, tag="ps")
                            for m in range(CD):
                                nc.tensor.matmul(
                                    ps[:, 0:384],
                                    lhsT=oT[:, m, t4 * 128:(t4 + 1) * 128],
                                    rhs=wo_sb[:, m, n * 384:(n + 1) * 384],
                                    start=(m == 0),
                                    stop=(m == CD - 1),
                                )
                            nc.vector.tensor_add(
                                out=ostage[:, n * 384:(n + 1) * 384],
                                in0=ps[:, 0:384],
                                in1=bo_bc[:, n * 384:(n + 1) * 384],
                            )
                        nc.sync.dma_start(
                            out=out_d.ap()[b, t4 * 128:(t4 + 1) * 128, :],
                            in_=ostage,
                        )
                    return emit
                return [t4_unit(t4) for t4 in range(4)]

            # ---- pipeline ----
            for unit in p1_units(0):
                unit()
            prev_p3 = []
            for b in range(BL):
                if b + 1 < BL:
                    emit_x_load(b + 1)
                    units = prev_p3 + p1_units(b + 1)
                else:
                    units = prev_p3
                emit_attention(b, units)
                for unit in units:
                    unit()
                prev_p3 = p3_units(b)
            for unit in prev_p3:
                unit()

    nc.finalize()
    return nc


_NC_CACHE = None


def _get_nc():
    global _NC_CACHE
    if _NC_CACHE is None:
        _NC_CACHE = build_nc()
    return _NC_CACHE


def run_spmd(inputs, trace=False, trace_cores=None):
    nc = _get_nc()
    x = np.ascontiguousarray(inputs["x"], dtype=np.float32)
    shared = {
        k: np.ascontiguousarray(inputs[k], dtype=np.float32)
        for k in ("Wq", "Wk", "Wv", "bq", "bk", "bv", "Wo", "bo")
    }
    in_maps = []
    for core in range(NCORES):
        m = dict(shared)
        m["x"] = np.ascontiguousarray(x[core * BL:(core + 1) * BL])
        in_maps.append(m)
    res = bass_utils.run_bass_kernel_spmd(
        nc, in_maps, core_ids=list(range(NCORES)),
        trace=trace, trace_cores=trace_cores,
    )
    return res


def kernel(**inputs) -> np.ndarray:
    res = run_spmd(inputs, trace=False)
    out = np.concatenate([res.results[i]["out"] for i in range(NCORES)], axis=0)
    return out.astype(np.float32)


# revision 37
# speedup vs baseline: 1.2202x; 1.0272x over previous
"""Multi-head attention (B=32, S=512, D=768, H=12, E=64) on 8 Trainium2 cores.

Sharding: data-parallel over batch — each of the 8 cores processes 4 batches
with a full copy of the weights. No collectives.

v2 design notes. Every engine queue is in-order, so emission order IS the
schedule; the kernel is laid out so the PE stream never waits:

  - Weights DMA'd tensor-by-tensor (Wq first) so Q-proj starts ~7us in
    instead of waiting ~26us for all weights.
  - Softmax denominator: pt tiles pair-summed on DVE (bf16; a Pool
    tensor_add costs ~1.7us on HW vs ~0.35us on DVE, and Pool ops ahead of
    DMA triggers in its queue delay the broadcast chain), r = two
    accumulating K=128 ones-matmuls (1 cyc/row), 1/r via
    reciprocal_approx_fast (DVE custom op, ~18 bits), then broadcast to all
    128 partitions by a DRAM-bounce on the sync HWDGE queue (write [1,S] +
    stride-0 partition re-read, f32). The normalize multiply is fused into
    the AV PSUM->SBUF eviction (one DVE mul per head) — no ScalarE copies.
    (gpsimd library ucode — partition_all_reduce etc. — is NOT available
    in this image (BEDROCK=1) and crashes the device; DVE TensorTensor
    divide is rejected by the ISA checker; HWDGE cannot cast in-flight.
    Hence this formulation.)
  - exp processes two key-blocks per instruction ([128,1024] PSUM tiles) to
    amortize ScalarE access-latency overhead.
  - The NEXT batch's transposes + QKV projections AND the PREVIOUS batch's
    out-projection are emitted as work units interleaved between attention
    heads of the current batch, filling the PE bubbles left by the
    scores->exp->AV dependency chain. Head pairs share one av PSUM tile
    (disjoint partition halves) to double the eviction lookahead.

Per-batch PE work: transposes 3072cy + QK 36864 + V 18432 + scores 24576 +
AV 24576 + r 10240 + out-proj 18432 ~= 136k cycles ~= 57us @2.4GHz;
everything else (ScalarE exp, DVE sums/recip/evictions, Pool sums/SWDGE)
fits underneath. Measured: ~350us vs 547us baseline (~1.55x), rel_err
5.26e-3 (unchanged from baseline).
"""

import numpy as np

import concourse.bass as bass
import concourse.tile as tile
import concourse.mybir as mybir
from concourse import bacc
from concourse import bass_utils
from concourse.masks import make_identity

B, S, D, H, E = 32, 512, 768, 12, 64
NCORES = 8
BL = B // NCORES          # local batches per core
CD = D // 128             # 6 chunks of 128 over d / he
F32 = mybir.dt.float32
BF16 = mybir.dt.bfloat16
AF = mybir.ActivationFunctionType


def build_nc():
    nc = bacc.Bacc(None)

    x_d = nc.dram_tensor("x", [BL, S, D], F32, kind="ExternalInput")
    wq_d = nc.dram_tensor("Wq", [H, D, E], F32, kind="ExternalInput")
    wk_d = nc.dram_tensor("Wk", [H, D, E], F32, kind="ExternalInput")
    wv_d = nc.dram_tensor("Wv", [H, D, E], F32, kind="ExternalInput")
    bq_d = nc.dram_tensor("bq", [H, E], F32, kind="ExternalInput")
    bk_d = nc.dram_tensor("bk", [H, E], F32, kind="ExternalInput")
    bv_d = nc.dram_tensor("bv", [H, E], F32, kind="ExternalInput")
    wo_d = nc.dram_tensor("Wo", [D, D], F32, kind="ExternalInput")
    bo_d = nc.dram_tensor("bo", [D], F32, kind="ExternalInput")
    out_d = nc.dram_tensor("out", [BL, S, D], F32, kind="ExternalOutput")
    # DRAM bounce buffer for the 1/r partition-broadcast (one slot per
    # (batch, head) so slots are never reused -> no WAR hazards)
    rscr_d = nc.dram_tensor("rscr", [BL * H, S], F32, kind="Internal")

    with nc.allow_low_precision(reason="bf16 intermediates"), \
         tile.TileContext(nc) as tc:
        with (
            tc.tile_pool(name="singles", bufs=1) as singles,
            tc.tile_pool(name="wstage", bufs=6) as wstage_pool,
            tc.tile_pool(name="xnat", bufs=3) as xnat_pool,
            tc.tile_pool(name="xt", bufs=2) as xt_pool,
            tc.tile_pool(name="qk", bufs=2) as qk_pool,
            tc.tile_pool(name="vv", bufs=2) as v_pool,
            tc.tile_pool(name="ot", bufs=2) as ot_pool,
            tc.tile_pool(name="pt", bufs=6) as pt_pool,
            tc.tile_pool(name="psm", bufs=4) as psum_pool,
            tc.tile_pool(name="rbc", bufs=2) as rbc_pool,
            tc.tile_pool(name="ostage", bufs=2) as out_pool,
            # PSUM budget (16KB/partition): pps 2x2KB + sc 2x4KB + av 2x2KB
            tc.tile_pool(name="pps", bufs=2, space="PSUM") as pps,
            tc.tile_pool(name="sc_ps", bufs=2, space="PSUM") as sc_ps,
            tc.tile_pool(name="av_ps", bufs=2, space="PSUM") as av_ps,
        ):
            # ---- constants ----
            ident = singles.tile([128, 128], BF16, tag="ident")
            make_identity(nc, ident)
            ones_col = singles.tile([128, 1], BF16, tag="onesc")
            nc.vector.memset(ones_col, 1.0)

            # ---- x(0) first (SWDGE f32->bf16 cast), then bias broadcasts
            # on the same queue ----
            xn_tiles = {}

            def emit_x_load(b):
                tl = []
                for t4 in range(4):
                    xn = xnat_pool.tile([128, D], BF16)
                    nc.gpsimd.dma_start(
                        out=xn, in_=x_d.ap()[b, t4 * 128:(t4 + 1) * 128, :]
                    )
                    tl.append(xn)
                xn_tiles[b] = tl

            emit_x_load(0)

            bv_bc = singles.tile([128, D], F32, tag="bvbc")
            bo_bc = singles.tile([128, D], F32, tag="bobc")
            for dst, src_d in ((bv_bc, bv_d), (bo_bc, bo_d)):
                f = src_d.ap().flatten()
                nc.gpsimd.dma_start(
                    out=dst,
                    in_=bass.AP(tensor=f.tensor, offset=f.offset,
                                ap=[[0, 128]] + [list(p) for p in f.ap]),
                )

            # ---- weights tensor-by-tensor; Wq + QK biases first so Q-proj
            # can start as soon as Wq lands ----
            w_sb = {}
            bq_sb = singles.tile([128, CD], F32, tag="bq")
            bk_sb = singles.tile([128, CD], F32, tag="bk")

            def load_w(name, wd, cast_fn):
                t = singles.tile([128, CD, D], BF16, tag=f"w{name}")
                src = wd.ap().rearrange("h (c p) e -> c p h e", p=128)
                for c in range(CD):
                    stg = wstage_pool.tile([128, D], F32)
                    nc.sync.dma_start(
                        out=stg.rearrange("p (h e) -> p h e", e=E), in_=src[c]
                    )
                    cast_fn(t[:, c, :], stg)
                w_sb[name] = t

            load_w("q", wq_d, lambda o, i: nc.vector.tensor_copy(out=o, in_=i))
            nc.sync.dma_start(
                out=bq_sb, in_=bq_d.ap().flatten().rearrange("(m p) -> p m", p=128)
            )
            nc.sync.dma_start(
                out=bk_sb, in_=bk_d.ap().flatten().rearrange("(m p) -> p m", p=128)
            )
            load_w("k", wk_d, lambda o, i: nc.vector.tensor_copy(out=o, in_=i))
            load_w("v", wv_d, lambda o, i: nc.gpsimd.tensor_copy(out=o, in_=i))
            wo_sb = singles.tile([128, CD, D], BF16, tag="wo")
            wo_src = wo_d.ap().rearrange("(c p) n -> c p n", p=128)
            for c in range(CD):
                stg = wstage_pool.tile([128, D], F32)
                nc.sync.dma_start(out=stg, in_=wo_src[c])
                nc.vector.tensor_copy(out=wo_sb[:, c, :], in_=stg)

            xt_tiles = {}
            qk_tiles = {}
            v_tiles = {}
            ot_tiles = {}

            def p1_units(b):
                """The next batch's prep as a list of PE work units, to be
                interleaved between attention heads of the current batch."""
                units = []
                xt = xt_pool.tile([128, CD, S], BF16)
                xt_tiles[b] = xt

                def transpose_unit(t4, cg, ncg):
                    def emit():
                        xn = xn_tiles[b][t4]
                        tp = pps.tile([128, S], BF16, tag="ps")
                        for j in range(ncg):
                            c = cg + j
                            nc.tensor.transpose(
                                tp[:, j * 128:(j + 1) * 128],
                                xn[:, c * 128:(c + 1) * 128],
                                ident,
                            )
                        nc.vector.tensor_copy(
                            out=xt[:, cg:cg + ncg, t4 * 128:(t4 + 1) * 128],
                            in_=tp[:, 0:ncg * 128].rearrange(
                                "p (c q) -> p c q", q=128),
                        )
                    return emit

                for t4 in range(4):
                    for cg, ncg in ((0, 4), (4, 2)):
                        units.append(transpose_unit(t4, cg, ncg))

                v_units = []
                v_sb = v_pool.tile([128, 4, D], BF16)
                v_tiles[b] = v_sb

                def v_unit(t4, n):
                    def emit():
                        ps = pps.tile([128, S], F32, tag="ps")
                        for c in range(CD):
                            nc.tensor.matmul(
                                ps[:, 0:384],
                                lhsT=xt[:, c, t4 * 128:(t4 + 1) * 128],
                                rhs=w_sb["v"][:, c, n * 384:(n + 1) * 384],
                                start=(c == 0),
                                stop=(c == CD - 1),
                            )
                        nc.vector.tensor_add(
                            out=v_sb[:, t4, n * 384:(n + 1) * 384],
                            in0=ps[:, 0:384],
                            in1=bv_bc[:, n * 384:(n + 1) * 384],
                        )
                    return emit

                for t4 in range(4):
                    for n in range(2):
                        v_units.append(v_unit(t4, n))

                qT = qk_pool.tile([128, CD, S], BF16, tag="qT")
                kT = qk_pool.tile([128, CD, S], BF16, tag="kT")
                qk_tiles[b] = (qT, kT)

                def qk_unit(dst, wname, bsb, m):
                    def emit():
                        ps = pps.tile([128, S], F32, tag="ps")
                        for c in range(CD):
                            nc.tensor.matmul(
                                ps,
                                lhsT=w_sb[wname][:, c, m * 128:(m + 1) * 128],
                                rhs=xt_tiles[b][:, c, :],
                                start=(c == 0),
                                stop=(c == CD - 1),
                            )
                        nc.scalar.add(
                            out=dst[:, m, :], in_=ps, add=bsb[:, m:m + 1],
                        )
                    return emit

                qk_units = [qk_unit(qT, "q", bq_sb, m) for m in range(CD)]
                qk_units += [qk_unit(kT, "k", bk_sb, m) for m in range(CD)]
                if b == 0:
                    # prologue: Wq lands first, Wk second, Wv third — order
                    # the GEMMs to chase the weight DMAs
                    units += qk_units + v_units
                else:
                    # steady state: V(t4) only needs its own t4 transposed;
                    # QK needs the full xt
                    units += v_units + qk_units
                return units

            def emit_attention(b, interleave):
                """Per head h: scores (2x [128,1024] psum) -> exp (ScalarE,
                two-block tiles) -> pair-sums (Pool+DVE, bf16) -> r via two
                accumulating ones-matmuls -> reciprocal_approx_fast -> 1/r
                broadcast to 128 partitions via a DRAM-bounce DMA (bf16 cast
                in flight) -> AV -> fused normalize-evict (DVE mul). Scores
                run 2 heads ahead; `interleave` units are popped between
                heads to fill PE bubbles."""
                qT, kT = qk_tiles[b]
                v_sb = v_tiles[b]
                oT = ot_pool.tile([128, CD, S], BF16, tag="oT")
                ot_tiles[b] = oT

                pt_tiles = {}
                sum_tiles = {}
                bc_tiles = {}

                def emit_scores(h):
                    hm, half = h // 2, 64 * (h % 2)
                    pts = []
                    for j in (0, 1):
                        sc = sc_ps.tile([128, 2, S], F32, tag="sc")
                        for i in (0, 1):
                            t4 = 2 * j + i
                            t4s = slice(t4 * 128, (t4 + 1) * 128)
                            nc.tensor.matmul(
                                sc[:, i, :],
                                lhsT=kT[half:half + 64, hm, t4s],
                                rhs=qT[half:half + 64, hm, :],
                                start=True, stop=True,
                            )
                        pt = pt_pool.tile([128, 2, S], BF16)
                        nc.scalar.activation(
                            out=pt.rearrange("p a b -> p (a b)"),
                            in_=sc.rearrange("p a b -> p (a b)"),
                            func=AF.Exp, scale=0.125)
                        pts.append(pt)
                    pt_tiles[h] = pts
                    s01 = psum_pool.tile([128, S], BF16, tag="s16")
                    nc.vector.tensor_add(out=s01, in0=pts[0][:, 0, :],
                                         in1=pts[0][:, 1, :])
                    s23 = psum_pool.tile([128, S], BF16, tag="s16")
                    nc.vector.tensor_add(out=s23, in0=pts[1][:, 0, :],
                                         in1=pts[1][:, 1, :])
                    sum_tiles[h] = (s01, s23)

                def emit_r(h):
                    """r + 1/r + broadcast, emitted one head ahead of AV so
                    the DMA bounce latency is covered by head h-1's AV."""
                    rp = pps.tile([128, S], F32, tag="ps")
                    ns = len(sum_tiles[h])
                    for j, s in enumerate(sum_tiles[h]):
                        nc.tensor.matmul(
                            rp[0:1, :], lhsT=ones_col, rhs=s,
                            start=(j == 0), stop=(j == ns - 1),
                        )
                    rinv = rbc_pool.tile([1, S], F32, tag="rinv")
                    nc.vector.reciprocal_approx_fast(out=rinv, in_=rp[0:1, :])
                    slot = rscr_d.ap()[b * H + h]
                    dout = nc.sync.dma_start(out=slot, in_=rinv)
                    bc = rbc_pool.tile([128, S], F32, tag="bc")
                    f = slot.flatten()
                    din = nc.sync.dma_start(
                        out=bc,
                        in_=bass.AP(tensor=f.tensor, offset=f.offset,
                                    ap=[[0, 128]] + [list(p) for p in f.ap]),
                    )
                    tile.add_dep_helper(din.ins, dout.ins, sync=True,
                                        reason="rinv bounce RAW")
                    bc_tiles[h] = bc
                    del sum_tiles[h]

                av_tiles = {}

                def emit_av(h):
                    hm, half = h // 2, 64 * (h % 2)
                    # head pairs share one av PSUM tile (disjoint partition
                    # halves) so evictions have twice the lookahead room
                    if h % 2 == 0:
                        av = av_ps.tile([128, S], F32, tag="av", name="av")
                        av_tiles[hm] = av
                    av = av_tiles[hm]
                    for t4 in range(4):
                        nc.tensor.matmul(
                            av[half:half + 64, :],
                            lhsT=v_sb[:, t4, h * 64:(h + 1) * 64],
                            rhs=pt_tiles[h][t4 // 2][:, t4 % 2, :],
                            start=(t4 == 0), stop=(t4 == 3),
                        )
                    # fused normalize + PSUM->SBUF eviction
                    nc.vector.tensor_mul(
                        out=oT[half:half + 64, hm, :],
                        in0=av[half:half + 64, :],
                        in1=bc_tiles[h][half:half + 64, :],
                    )
                    del pt_tiles[h], bc_tiles[h]

                emit_scores(0)
                emit_scores(1)
                # batch start: no AV yet to cover the first r chain — pop a
                # couple of interleave units so the PE isn't waiting on exp
                for _ in range(2):
                    if interleave:
                        interleave.pop(0)()
                emit_r(0)
                for h in range(H):
                    if h + 2 < H:
                        emit_scores(h + 2)
                    if h + 1 < H:
                        emit_r(h + 1)
                    emit_av(h)
                    for _ in range(2):
                        if interleave:
                            interleave.pop(0)()

            def p3_units(b):
                """Out-projection as 4 per-token-block units, interleaved
                into the NEXT batch's attention for PE cover."""
                def t4_unit(t4):
                    def emit():
                        oT = ot_tiles[b]
                        ostage = out_pool.tile([128, D], F32)
                        for n in range(2):
                            ps = pps.tile([128, S], F32, tag="ps")
                            for m in range(CD):
                                nc.tensor.matmul(
                                    ps[:, 0:384],
                                    lhsT=oT[:, m, t4 * 128:(t4 + 1) * 128],
                                    rhs=wo_sb[:, m, n * 384:(n + 1) * 384],
                                    start=(m == 0),
                                    stop=(m == CD - 1),
                                )
                            nc.vector.tensor_add(
                                out=ostage[:, n * 384:(n + 1) * 384],
                                in0=ps[:, 0:384],
                                in1=bo_bc[:, n * 384:(n + 1) * 384],
                            )
                        nc.sync.dma_start(
                            out=out_d.ap()[b, t4 * 128:(t4 + 1) * 128, :],
                            in_=ostage,
                        )
                    return emit
                return [t4_unit(t4) for t4 in range(4)]

            # ---- pipeline ----
            for unit in p1_units(0):
                unit()
            prev_p3 = []
            for b in range(BL):
                if b + 1 < BL:
                    emit_x_load(b + 1)
                    units = prev_p3 + p1_units(b + 1)
                else:
                    units = prev_p3
                emit_attention(b, units)
                for unit in units:
                    unit()
                prev_p3 = p3_units(b)
            for unit in prev_p3:
                unit()

    nc.finalize()
    return nc


_NC_CACHE = None


def _get_nc():
    global _NC_CACHE
    if _NC_CACHE is None:
        _NC_CACHE = build_nc()
    return _NC_CACHE


def run_spmd(inputs, trace=False, trace_cores=None):
    nc = _get_nc()
    x = np.ascontiguousarray(inputs["x"], dtype=np.float32)
    shared = {
        k: np.ascontiguousarray(inputs[k], dtype=np.float32)
        for k in ("Wq", "Wk", "Wv", "bq", "bk", "bv", "Wo", "bo")
    }
    in_maps = []
    for core in range(NCORES):
        m = dict(shared)
        m["x"] = np.ascontiguousarray(x[core * BL:(core + 1) * BL])
        in_maps.append(m)
    res = bass_utils.run_bass_kernel_spmd(
        nc, in_maps, core_ids=list(range(NCORES)),
        trace=trace, trace_cores=trace_cores,
    )
    return res


def kernel(**inputs) -> np.ndarray:
    res = run_spmd(inputs, trace=False)
    out = np.concatenate([res.results[i]["out"] for i in range(NCORES)], axis=0)
    return out.astype(np.float32)


# revision 38
# speedup vs baseline: 1.2248x; 1.0037x over previous
"""Multi-head attention (B=32, S=512, D=768, H=12, E=64) on 8 Trainium2 cores.

Sharding: data-parallel over batch — each of the 8 cores processes 4 batches
with a full copy of the weights. No collectives.

v2 design notes. Every engine queue is in-order, so emission order IS the
schedule; the kernel is laid out so the PE stream never waits:

  - Weights DMA'd tensor-by-tensor (Wq first) so Q-proj starts ~7us in
    instead of waiting ~26us for all weights.
  - Softmax denominator: pt tiles pair-summed on DVE (bf16; a Pool
    tensor_add costs ~1.7us on HW vs ~0.35us on DVE, and Pool ops ahead of
    DMA triggers in its queue delay the broadcast chain), r = two
    accumulating K=128 ones-matmuls (1 cyc/row), 1/r via
    reciprocal_approx_fast (DVE custom op, ~18 bits), then broadcast to all
    128 partitions by a DRAM-bounce on the sync HWDGE queue (write [1,S] +
    stride-0 partition re-read, f32). The normalize multiply is fused into
    the AV PSUM->SBUF eviction (one DVE mul per head) — no ScalarE copies.
    (gpsimd library ucode — partition_all_reduce etc. — is NOT available
    in this image (BEDROCK=1) and crashes the device; DVE TensorTensor
    divide is rejected by the ISA checker; HWDGE cannot cast in-flight.
    Hence this formulation.)
  - exp processes two key-blocks per instruction ([128,1024] PSUM tiles) to
    amortize ScalarE access-latency overhead.
  - The NEXT batch's transposes + QKV projections AND the PREVIOUS batch's
    out-projection are emitted as work units interleaved between attention
    heads of the current batch, filling the PE bubbles left by the
    scores->exp->AV dependency chain. Head pairs share one av PSUM tile
    (disjoint partition halves) to double the eviction lookahead.

Per-batch PE work: transposes 3072cy + QK 36864 + V 18432 + scores 24576 +
AV 24576 + r 10240 + out-proj 18432 ~= 136k cycles ~= 57us @2.4GHz;
everything else (ScalarE exp, DVE sums/recip/evictions, Pool SWDGE) fits
underneath. Measured: ~340us vs 547us baseline (~1.6x; note ~10% run-to-run
DVFS drift on the dev machine), rel_err 5.01e-3. TimelineSim predicts 279us.
"""

import numpy as np

import concourse.bass as bass
import concourse.tile as tile
import concourse.mybir as mybir
from concourse import bacc
from concourse import bass_utils
from concourse.masks import make_identity

B, S, D, H, E = 32, 512, 768, 12, 64
NCORES = 8
BL = B // NCORES          # local batches per core
CD = D // 128             # 6 chunks of 128 over d / he
F32 = mybir.dt.float32
BF16 = mybir.dt.bfloat16
AF = mybir.ActivationFunctionType


def build_nc():
    nc = bacc.Bacc(None)

    x_d = nc.dram_tensor("x", [BL, S, D], F32, kind="ExternalInput")
    wq_d = nc.dram_tensor("Wq", [H, D, E], F32, kind="ExternalInput")
    wk_d = nc.dram_tensor("Wk", [H, D, E], F32, kind="ExternalInput")
    wv_d = nc.dram_tensor("Wv", [H, D, E], F32, kind="ExternalInput")
    bq_d = nc.dram_tensor("bq", [H, E], F32, kind="ExternalInput")
    bk_d = nc.dram_tensor("bk", [H, E], F32, kind="ExternalInput")
    bv_d = nc.dram_tensor("bv", [H, E], F32, kind="ExternalInput")
    wo_d = nc.dram_tensor("Wo", [D, D], F32, kind="ExternalInput")
    bo_d = nc.dram_tensor("bo", [D], F32, kind="ExternalInput")
    out_d = nc.dram_tensor("out", [BL, S, D], F32, kind="ExternalOutput")
    # DRAM bounce buffer for the 1/r partition-broadcast (one slot per
    # (batch, head) so slots are never reused -> no WAR hazards)
    rscr_d = nc.dram_tensor("rscr", [BL * H, S], F32, kind="Internal")

    with nc.allow_low_precision(reason="bf16 intermediates"), \
         tile.TileContext(nc) as tc:
        with (
            tc.tile_pool(name="singles", bufs=1) as singles,
            tc.tile_pool(name="wstage", bufs=6) as wstage_pool,
            tc.tile_pool(name="xnat", bufs=3) as xnat_pool,
            tc.tile_pool(name="xt", bufs=2) as xt_pool,
            tc.tile_pool(name="qk", bufs=2) as qk_pool,
            tc.tile_pool(name="vv", bufs=2) as v_pool,
            tc.tile_pool(name="ot", bufs=2) as ot_pool,
            tc.tile_pool(name="pt", bufs=6) as pt_pool,
            tc.tile_pool(name="psm", bufs=4) as psum_pool,
            tc.tile_pool(name="rbc", bufs=2) as rbc_pool,
            tc.tile_pool(name="ostage", bufs=2) as out_pool,
            # PSUM budget (16KB/partition): pps 2x2KB + sc 2x4KB + av 2x2KB
            tc.tile_pool(name="pps", bufs=2, space="PSUM") as pps,
            tc.tile_pool(name="sc_ps", bufs=2, space="PSUM") as sc_ps,
            tc.tile_pool(name="av_ps", bufs=2, space="PSUM") as av_ps,
        ):
            # ---- constants ----
            ident = singles.tile([128, 128], BF16, tag="ident")
            make_identity(nc, ident)
            ones_col = singles.tile([128, 1], BF16, tag="onesc")
            nc.vector.memset(ones_col, 1.0)

            # ---- x(0) first (SWDGE f32->bf16 cast), then bias broadcasts
            # on the same queue ----
            xn_tiles = {}

            def emit_x_load(b):
                tl = []
                for t4 in range(4):
                    xn = xnat_pool.tile([128, D], BF16)
                    nc.gpsimd.dma_start(
                        out=xn, in_=x_d.ap()[b, t4 * 128:(t4 + 1) * 128, :]
                    )
                    tl.append(xn)
                xn_tiles[b] = tl

            emit_x_load(0)

            bv_bc = singles.tile([128, D], F32, tag="bvbc")
            bo_bc = singles.tile([128, D], F32, tag="bobc")
            for dst, src_d in ((bv_bc, bv_d), (bo_bc, bo_d)):
                f = src_d.ap().flatten()
                nc.gpsimd.dma_start(
                    out=dst,
                    in_=bass.AP(tensor=f.tensor, offset=f.offset,
                                ap=[[0, 128]] + [list(p) for p in f.ap]),
                )

            # ---- weights tensor-by-tensor; Wq + QK biases first so Q-proj
            # can start as soon as Wq lands ----
            w_sb = {}
            bq_sb = singles.tile([128, CD], F32, tag="bq")
            bk_sb = singles.tile([128, CD], F32, tag="bk")

            def load_w(name, wd, cast_fn):
                t = singles.tile([128, CD, D], BF16, tag=f"w{name}")
                src = wd.ap().rearrange("h (c p) e -> c p h e", p=128)
                for c in range(CD):
                    stg = wstage_pool.tile([128, D], F32)
                    nc.sync.dma_start(
                        out=stg.rearrange("p (h e) -> p h e", e=E), in_=src[c]
                    )
                    cast_fn(t[:, c, :], stg)
                w_sb[name] = t

            load_w("q", wq_d, lambda o, i: nc.vector.tensor_copy(out=o, in_=i))
            nc.sync.dma_start(
                out=bq_sb, in_=bq_d.ap().flatten().rearrange("(m p) -> p m", p=128)
            )
            nc.sync.dma_start(
                out=bk_sb, in_=bk_d.ap().flatten().rearrange("(m p) -> p m", p=128)
            )
            load_w("k", wk_d, lambda o, i: nc.vector.tensor_copy(out=o, in_=i))
            load_w("v", wv_d, lambda o, i: nc.gpsimd.tensor_copy(out=o, in_=i))
            wo_sb = singles.tile([128, CD, D], BF16, tag="wo")
            wo_src = wo_d.ap().rearrange("(c p) n -> c p n", p=128)
            for c in range(CD):
                stg = wstage_pool.tile([128, D], F32)
                nc.sync.dma_start(out=stg, in_=wo_src[c])
                nc.vector.tensor_copy(out=wo_sb[:, c, :], in_=stg)

            xt_tiles = {}
            qk_tiles = {}
            v_tiles = {}
            ot_tiles = {}

            def p1_units(b):
                """The next batch's prep as a list of PE work units, to be
                interleaved between attention heads of the current batch."""
                units = []
                xt = xt_pool.tile([128, CD, S], BF16)
                xt_tiles[b] = xt

                def transpose_unit(t4, cg, ncg):
                    def emit():
                        xn = xn_tiles[b][t4]
                        tp = pps.tile([128, S], BF16, tag="ps")
                        for j in range(ncg):
                            c = cg + j
                            nc.tensor.transpose(
                                tp[:, j * 128:(j + 1) * 128],
                                xn[:, c * 128:(c + 1) * 128],
                                ident,
                            )
                        nc.vector.tensor_copy(
                            out=xt[:, cg:cg + ncg, t4 * 128:(t4 + 1) * 128],
                            in_=tp[:, 0:ncg * 128].rearrange(
                                "p (c q) -> p c q", q=128),
                        )
                    return emit

                for t4 in range(4):
                    for cg, ncg in ((0, 4), (4, 2)):
                        units.append(transpose_unit(t4, cg, ncg))

                v_units = []
                v_sb = v_pool.tile([128, 4, D], BF16)
                v_tiles[b] = v_sb

                def v_unit(t4, n):
                    def emit():
                        ps = pps.tile([128, S], F32, tag="ps")
                        for c in range(CD):
                            nc.tensor.matmul(
                                ps[:, 0:384],
                                lhsT=xt[:, c, t4 * 128:(t4 + 1) * 128],
                                rhs=w_sb["v"][:, c, n * 384:(n + 1) * 384],
                                start=(c == 0),
                                stop=(c == CD - 1),
                            )
                        nc.vector.tensor_add(
                            out=v_sb[:, t4, n * 384:(n + 1) * 384],
                            in0=ps[:, 0:384],
                            in1=bv_bc[:, n * 384:(n + 1) * 384],
                        )
                    return emit

                for t4 in range(4):
                    for n in range(2):
                        v_units.append(v_unit(t4, n))

                qT = qk_pool.tile([128, CD, S], BF16, tag="qT")
                kT = qk_pool.tile([128, CD, S], BF16, tag="kT")
                qk_tiles[b] = (qT, kT)

                def qk_unit(dst, wname, bsb, m):
                    def emit():
                        ps = pps.tile([128, S], F32, tag="ps")
                        for c in range(CD):
                            nc.tensor.matmul(
                                ps,
                                lhsT=w_sb[wname][:, c, m * 128:(m + 1) * 128],
                                rhs=xt_tiles[b][:, c, :],
                                start=(c == 0),
                                stop=(c == CD - 1),
                            )
                        nc.scalar.add(
                            out=dst[:, m, :], in_=ps, add=bsb[:, m:m + 1],
                        )
                    return emit

                qk_units = [qk_unit(qT, "q", bq_sb, m) for m in range(CD)]
                qk_units += [qk_unit(kT, "k", bk_sb, m) for m in range(CD)]
                if b == 0:
                    # prologue: Wq lands first, Wk second, Wv third — order
                    # the GEMMs to chase the weight DMAs
                    units += qk_units + v_units
                else:
                    # steady state: V(t4) only needs its own t4 transposed;
                    # QK needs the full xt
                    units += v_units + qk_units
                return units

            def emit_attention(b, interleave):
                """Per head h: scores (2x [128,1024] psum) -> exp (ScalarE,
                two-block tiles) -> pair-sums (Pool+DVE, bf16) -> r via two
                accumulating ones-matmuls -> reciprocal_approx_fast -> 1/r
                broadcast to 128 partitions via a DRAM-bounce DMA (bf16 cast
                in flight) -> AV -> fused normalize-evict (DVE mul). Scores
                run 2 heads ahead; `interleave` units are popped between
                heads to fill PE bubbles."""
                qT, kT = qk_tiles[b]
                v_sb = v_tiles[b]
                oT = ot_pool.tile([128, CD, S], BF16, tag="oT")
                ot_tiles[b] = oT

                pt_tiles = {}
                sum_tiles = {}
                bc_tiles = {}

                def emit_scores(h):
                    hm, half = h // 2, 64 * (h % 2)
                    pts = []
                    for j in (0, 1):
                        sc = sc_ps.tile([128, 2, S], F32, tag="sc")
                        for i in (0, 1):
                            t4 = 2 * j + i
                            t4s = slice(t4 * 128, (t4 + 1) * 128)
                            nc.tensor.matmul(
                                sc[:, i, :],
                                lhsT=kT[half:half + 64, hm, t4s],
                                rhs=qT[half:half + 64, hm, :],
                                start=True, stop=True,
                            )
                        pt = pt_pool.tile([128, 2, S], BF16)
                        nc.scalar.activation(
                            out=pt.rearrange("p a b -> p (a b)"),
                            in_=sc.rearrange("p a b -> p (a b)"),
                            func=AF.Exp, scale=0.125)
                        pts.append(pt)
                    pt_tiles[h] = pts
                    s01 = psum_pool.tile([128, S], BF16, tag="s16")
                    nc.vector.tensor_add(out=s01, in0=pts[0][:, 0, :],
                                         in1=pts[0][:, 1, :])
                    s23 = psum_pool.tile([128, S], BF16, tag="s16")
                    nc.vector.tensor_add(out=s23, in0=pts[1][:, 0, :],
                                         in1=pts[1][:, 1, :])
                    sum_tiles[h] = (s01, s23)

                def emit_r(h):
                    """r + 1/r + broadcast, emitted one head ahead of AV so
                    the DMA bounce latency is covered by head h-1's AV."""
                    rp = pps.tile([128, S], F32, tag="ps")
                    ns = len(sum_tiles[h])
                    for j, s in enumerate(sum_tiles[h]):
                        nc.tensor.matmul(
                            rp[0:1, :], lhsT=ones_col, rhs=s,
                            start=(j == 0), stop=(j == ns - 1),
                        )
                    rinv = rbc_pool.tile([1, S], F32, tag="rinv")
                    nc.vector.reciprocal_approx_fast(out=rinv, in_=rp[0:1, :])
                    slot = rscr_d.ap()[b * H + h]
                    dout = nc.sync.dma_start(out=slot, in_=rinv)
                    bc = rbc_pool.tile([128, S], F32, tag="bc")
                    f = slot.flatten()
                    din = nc.sync.dma_start(
                        out=bc,
                        in_=bass.AP(tensor=f.tensor, offset=f.offset,
                                    ap=[[0, 128]] + [list(p) for p in f.ap]),
                    )
                    tile.add_dep_helper(din.ins, dout.ins, sync=True,
                                        reason="rinv bounce RAW")
                    bc_tiles[h] = bc
                    del sum_tiles[h]

                av_tiles = {}

                def emit_av(h):
                    hm, half = h // 2, 64 * (h % 2)
                    # head pairs share one av PSUM tile (disjoint partition
                    # halves) so evictions have twice the lookahead room
                    if h % 2 == 0:
                        av = av_ps.tile([128, S], F32, tag="av", name="av")
                        av_tiles[hm] = av
                    av = av_tiles[hm]
                    for t4 in range(4):
                        nc.tensor.matmul(
                            av[half:half + 64, :],
                            lhsT=v_sb[:, t4, h * 64:(h + 1) * 64],
                            rhs=pt_tiles[h][t4 // 2][:, t4 % 2, :],
                            start=(t4 == 0), stop=(t4 == 3),
                        )
                    # fused normalize + PSUM->SBUF eviction
                    nc.vector.tensor_mul(
                        out=oT[half:half + 64, hm, :],
                        in0=av[half:half + 64, :],
                        in1=bc_tiles[h][half:half + 64, :],
                    )
                    del pt_tiles[h], bc_tiles[h]

                emit_scores(0)
                emit_scores(1)
                # batch start: no AV yet to cover the first r chain — pop a
                # couple of interleave units so the PE isn't waiting on exp
                for _ in range(2):
                    if interleave:
                        interleave.pop(0)()
                emit_r(0)
                for h in range(H):
                    if h + 2 < H:
                        emit_scores(h + 2)
                    if h + 1 < H:
                        emit_r(h + 1)
                    emit_av(h)
                    for _ in range(2):
                        if interleave:
                            interleave.pop(0)()

            def p3_units(b):
                """Out-projection as 4 per-token-block units, interleaved
                into the NEXT batch's attention for PE cover."""
                def t4_unit(t4):
                    def emit():
                        oT = ot_tiles[b]
                        ostage = out_pool.tile([128, D], F32)
                        for n in range(2):
                            ps = pps.tile([128, S], F32, tag="ps")
                            for m in range(CD):
                                nc.tensor.matmul(
                                    ps[:, 0:384],
                                    lhsT=oT[:, m, t4 * 128:(t4 + 1) * 128],
                                    rhs=wo_sb[:, m, n * 384:(n + 1) * 384],
                                    start=(m == 0),
                                    stop=(m == CD - 1),
                                )
                            nc.vector.tensor_add(
                                out=ostage[:, n * 384:(n + 1) * 384],
                                in0=ps[:, 0:384],
                                in1=bo_bc[:, n * 384:(n + 1) * 384],
                            )
                        nc.sync.dma_start(
                            out=out_d.ap()[b, t4 * 128:(t4 + 1) * 128, :],
                            in_=ostage,
                        )
                    return emit
                return [t4_unit(t4) for t4 in range(4)]

            # ---- pipeline ----
            for unit in p1_units(0):
                unit()
            prev_p3 = []
            for b in range(BL):
                if b + 1 < BL:
                    emit_x_load(b + 1)
                    units = prev_p3 + p1_units(b + 1)
                else:
                    units = prev_p3
                emit_attention(b, units)
                for unit in units:
                    unit()
                prev_p3 = p3_units(b)
            for unit in prev_p3:
                unit()

    nc.finalize()
    return nc


_NC_CACHE = None


def _get_nc():
    global _NC_CACHE
    if _NC_CACHE is None:
        _NC_CACHE = build_nc()
    return _NC_CACHE


def run_spmd(inputs, trace=False, trace_cores=None):
    nc = _get_nc()
    x = np.ascontiguousarray(inputs["x"], dtype=np.float32)
    shared = {
        k: np.ascontiguousarray(inputs[k], dtype=np.float32)
        for k in ("Wq", "Wk", "Wv", "bq", "bk", "bv", "Wo", "bo")
    }
    in_maps = []
    for core in range(NCORES):
        m = dict(shared)
        m["x"] = np.ascontiguousarray(x[core * BL:(core + 1) * BL])
        in_maps.append(m)
    res = bass_utils.run_bass_kernel_spmd(
        nc, in_maps, core_ids=list(range(NCORES)),
        trace=trace, trace_cores=trace_cores,
    )
    return res


def kernel(**inputs) -> np.ndarray:
    res = run_spmd(inputs, trace=False)
    out = np.concatenate([res.results[i]["out"] for i in range(NCORES)], axis=0)
    return out.astype(np.float32)


# revision 41
# speedup vs baseline: 1.2572x; 1.0265x over previous
"""Multi-head attention (B=32, S=512, D=768, H=12, E=64) on 8 Trainium2 cores.

Sharding: data-parallel over batch — each of the 8 cores processes 4 batches
with a full copy of the weights. No collectives.

v2 design notes. Every engine queue is in-order, so emission order IS the
schedule; the kernel is laid out so the PE stream never waits:

  - Weights DMA'd tensor-by-tensor (Wq first) so Q-proj starts ~7us in
    instead of waiting ~26us for all weights.
  - Softmax denominator: pt tiles pair-summed on DVE (bf16; a Pool
    tensor_add costs ~1.7us on HW vs ~0.35us on DVE, and Pool ops ahead of
    DMA triggers in its queue delay the broadcast chain), r = two
    accumulating K=128 ones-matmuls (1 cyc/row), 1/r via
    reciprocal_approx_fast (DVE custom op, ~18 bits), then broadcast to all
    128 partitions by a DRAM-bounce on the sync HWDGE queue (write [1,S] +
    stride-0 partition re-read, f32). The normalize multiply is fused into
    the AV PSUM->SBUF eviction (one DVE mul per head) — no ScalarE copies.
    (gpsimd library ucode — partition_all_reduce etc. — is NOT available
    in this image (BEDROCK=1) and crashes the device; DVE TensorTensor
    divide is rejected by the ISA checker; HWDGE cannot cast in-flight.
    Hence this formulation.)
  - exp processes two key-blocks per instruction ([128,1024] PSUM tiles) to
    amortize ScalarE access-latency overhead.
  - The NEXT batch's transposes + QKV projections AND the PREVIOUS batch's
    out-projection are emitted as work units interleaved between attention
    heads of the current batch, filling the PE bubbles left by the
    scores->exp->AV dependency chain. Head pairs share one av PSUM tile
    (disjoint partition halves) to double the eviction lookahead.

Per-batch PE work: transposes 3072cy + QK 36864 + V 18432 + scores 24576 +
AV 24576 + r 10240 + out-proj 18432 ~= 136k cycles ~= 57us @2.4GHz;
everything else (ScalarE exp, DVE sums/recip/evictions, Pool SWDGE) fits
underneath. Measured: ~340us vs 547us baseline (~1.6x; note ~10% run-to-run
DVFS drift on the dev machine), rel_err 5.01e-3. TimelineSim predicts 279us.
"""

import numpy as np

import concourse.bass as bass
import concourse.tile as tile
import concourse.mybir as mybir
from concourse import bacc
from concourse import bass_utils
from concourse.masks import make_identity

B, S, D, H, E = 32, 512, 768, 12, 64
NCORES = 8
BL = B // NCORES          # local batches per core
CD = D // 128             # 6 chunks of 128 over d / he
F32 = mybir.dt.float32
BF16 = mybir.dt.bfloat16
AF = mybir.ActivationFunctionType


def build_nc():
    nc = bacc.Bacc(None)

    x_d = nc.dram_tensor("x", [BL, S, D], F32, kind="ExternalInput")
    wq_d = nc.dram_tensor("Wq", [H, D, E], F32, kind="ExternalInput")
    wk_d = nc.dram_tensor("Wk", [H, D, E], F32, kind="ExternalInput")
    wv_d = nc.dram_tensor("Wv", [H, D, E], F32, kind="ExternalInput")
    bq_d = nc.dram_tensor("bq", [H, E], F32, kind="ExternalInput")
    bk_d = nc.dram_tensor("bk", [H, E], F32, kind="ExternalInput")
    bv_d = nc.dram_tensor("bv", [H, E], F32, kind="ExternalInput")
    wo_d = nc.dram_tensor("Wo", [D, D], F32, kind="ExternalInput")
    bo_d = nc.dram_tensor("bo", [D], F32, kind="ExternalInput")
    out_d = nc.dram_tensor("out", [BL, S, D], F32, kind="ExternalOutput")
    # DRAM bounce buffer for the 1/r partition-broadcast (one slot per
    # (batch, head) so slots are never reused -> no WAR hazards)
    rscr_d = nc.dram_tensor("rscr", [BL * H, S], F32, kind="Internal")

    with nc.allow_low_precision(reason="bf16 intermediates"), \
         tile.TileContext(nc) as tc:
        with (
            tc.tile_pool(name="singles", bufs=1) as singles,
            tc.tile_pool(name="wstage", bufs=6) as wstage_pool,
            tc.tile_pool(name="xnat", bufs=3) as xnat_pool,
            tc.tile_pool(name="xt", bufs=2) as xt_pool,
            tc.tile_pool(name="qk", bufs=2) as qk_pool,
            tc.tile_pool(name="vv", bufs=2) as v_pool,
            tc.tile_pool(name="ot", bufs=2) as ot_pool,
            tc.tile_pool(name="pt", bufs=6) as pt_pool,
            tc.tile_pool(name="psm", bufs=4) as psum_pool,
            tc.tile_pool(name="rbc", bufs=2) as rbc_pool,
            tc.tile_pool(name="ostage", bufs=2) as out_pool,
            # PSUM budget (16KB/partition): pps 2x2KB + sc 2x4KB + av 2x2KB
            tc.tile_pool(name="pps", bufs=2, space="PSUM") as pps,
            tc.tile_pool(name="sc_ps", bufs=2, space="PSUM") as sc_ps,
            tc.tile_pool(name="av_ps", bufs=2, space="PSUM") as av_ps,
        ):
            # ---- constants ----
            ident = singles.tile([128, 128], BF16, tag="ident")
            make_identity(nc, ident)
            ones_col = singles.tile([128, 1], BF16, tag="onesc")
            nc.vector.memset(ones_col, 1.0)

            # ---- x(0) first (SWDGE f32->bf16 cast), then bias broadcasts
            # on the same queue ----
            xn_tiles = {}

            def emit_x_load(b):
                tl = []
                for t4 in range(4):
                    xn = xnat_pool.tile([128, D], BF16)
                    nc.gpsimd.dma_start(
                        out=xn, in_=x_d.ap()[b, t4 * 128:(t4 + 1) * 128, :]
                    )
                    tl.append(xn)
                xn_tiles[b] = tl

            emit_x_load(0)

            bv_bc = singles.tile([128, D], F32, tag="bvbc")
            bo_bc = singles.tile([128, D], F32, tag="bobc")
            for dst, src_d in ((bv_bc, bv_d), (bo_bc, bo_d)):
                f = src_d.ap().flatten()
                nc.gpsimd.dma_start(
                    out=dst,
                    in_=bass.AP(tensor=f.tensor, offset=f.offset,
                                ap=[[0, 128]] + [list(p) for p in f.ap]),
                )

            # ---- weights tensor-by-tensor; Wq + QK biases first so Q-proj
            # can start as soon as Wq lands ----
            w_sb = {}
            bq_sb = singles.tile([128, CD], F32, tag="bq")
            bk_sb = singles.tile([128, CD], F32, tag="bk")

            def load_w(name, wd, cast_fn):
                t = singles.tile([128, CD, D], BF16, tag=f"w{name}")
                src = wd.ap().rearrange("h (c p) e -> c p h e", p=128)
                for c in range(CD):
                    stg = wstage_pool.tile([128, D], F32)
                    nc.sync.dma_start(
                        out=stg.rearrange("p (h e) -> p h e", e=E), in_=src[c]
                    )
                    cast_fn(t[:, c, :], stg)
                w_sb[name] = t

            load_w("q", wq_d, lambda o, i: nc.vector.tensor_copy(out=o, in_=i))
            nc.sync.dma_start(
                out=bq_sb, in_=bq_d.ap().flatten().rearrange("(m p) -> p m", p=128)
            )
            nc.sync.dma_start(
                out=bk_sb, in_=bk_d.ap().flatten().rearrange("(m p) -> p m", p=128)
            )
            load_w("k", wk_d, lambda o, i: nc.vector.tensor_copy(out=o, in_=i))
            load_w("v", wv_d, lambda o, i: nc.gpsimd.tensor_copy(out=o, in_=i))
            wo_sb = singles.tile([128, CD, D], BF16, tag="wo")
            wo_src = wo_d.ap().rearrange("(c p) n -> c p n", p=128)
            for c in range(CD):
                stg = wstage_pool.tile([128, D], F32)
                nc.sync.dma_start(out=stg, in_=wo_src[c])
                nc.vector.tensor_copy(out=wo_sb[:, c, :], in_=stg)

            xt_tiles = {}
            qk_tiles = {}
            v_tiles = {}
            ot_tiles = {}

            def p1_units(b):
                """The next batch's prep as a list of PE work units, to be
                interleaved between attention heads of the current batch."""
                units = []
                xt = xt_pool.tile([128, CD, S], BF16)
                xt_tiles[b] = xt

                def transpose_unit(t4, cg, ncg):
                    def emit():
                        xn = xn_tiles[b][t4]
                        tp = pps.tile([128, S], BF16, tag="ps")
                        for j in range(ncg):
                            c = cg + j
                            nc.tensor.transpose(
                                tp[:, j * 128:(j + 1) * 128],
                                xn[:, c * 128:(c + 1) * 128],
                                ident,
                            )
                        nc.vector.tensor_copy(
                            out=xt[:, cg:cg + ncg, t4 * 128:(t4 + 1) * 128],
                            in_=tp[:, 0:ncg * 128].rearrange(
                                "p (c q) -> p c q", q=128),
                        )
                    return emit

                for t4 in range(4):
                    for cg, ncg in ((0, 4), (4, 2)):
                        units.append(transpose_unit(t4, cg, ncg))

                v_units = []
                v_sb = v_pool.tile([128, 4, D], BF16)
                v_tiles[b] = v_sb

                def v_unit(t4, n):
                    def emit():
                        ps = pps.tile([128, S], F32, tag="ps")
                        for c in range(CD):
                            nc.tensor.matmul(
                                ps[:, 0:384],
                                lhsT=xt[:, c, t4 * 128:(t4 + 1) * 128],
                                rhs=w_sb["v"][:, c, n * 384:(n + 1) * 384],
                                start=(c == 0),
                                stop=(c == CD - 1),
                            )
                        nc.vector.tensor_add(
                            out=v_sb[:, t4, n * 384:(n + 1) * 384],
                            in0=ps[:, 0:384],
                            in1=bv_bc[:, n * 384:(n + 1) * 384],
                        )
                    return emit

                for t4 in range(4):
                    for n in range(2):
                        v_units.append(v_unit(t4, n))

                qT = qk_pool.tile([128, CD, S], BF16, tag="qT")
                kT = qk_pool.tile([128, CD, S], BF16, tag="kT")
                qk_tiles[b] = (qT, kT)

                def qk_unit(dst, wname, bsb, m):
                    def emit():
                        ps = pps.tile([128, S], F32, tag="ps")
                        for c in range(CD):
                            nc.tensor.matmul(
                                ps,
                                lhsT=w_sb[wname][:, c, m * 128:(m + 1) * 128],
                                rhs=xt_tiles[b][:, c, :],
                                start=(c == 0),
                                stop=(c == CD - 1),
                            )
                        nc.scalar.add(
                            out=dst[:, m, :], in_=ps, add=bsb[:, m:m + 1],
                        )
                    return emit

                qk_units = [qk_unit(qT, "q", bq_sb, m) for m in range(CD)]
                qk_units += [qk_unit(kT, "k", bk_sb, m) for m in range(CD)]
                if b == 0:
                    # prologue: Wq lands first, Wk second, Wv third — order
                    # the GEMMs to chase the weight DMAs
                    units += qk_units + v_units
                else:
                    # steady state: V(t4) only needs its own t4 transposed;
                    # QK needs the full xt
                    units += v_units + qk_units
                return units

            def emit_attention(b, interleave):
                """Per head h: scores (2x [128,1024] psum) -> exp (ScalarE,
                two-block tiles) -> pair-sums (Pool+DVE, bf16) -> r via two
                accumulating ones-matmuls -> reciprocal_approx_fast -> 1/r
                broadcast to 128 partitions via a DRAM-bounce DMA (bf16 cast
                in flight) -> AV -> fused normalize-evict (DVE mul). Scores
                run 2 heads ahead; `interleave` units are popped between
                heads to fill PE bubbles."""
                qT, kT = qk_tiles[b]
                v_sb = v_tiles[b]
                oT = ot_pool.tile([128, CD, S], BF16, tag="oT")
                ot_tiles[b] = oT

                pt_tiles = {}
                sum_tiles = {}
                bc_tiles = {}

                def emit_scores(h):
                    hm, half = h // 2, 64 * (h % 2)
                    pts = []
                    for j in (0, 1):
                        sc = sc_ps.tile([128, 2, S], F32, tag="sc")
                        for i in (0, 1):
                            t4 = 2 * j + i
                            t4s = slice(t4 * 128, (t4 + 1) * 128)
                            nc.tensor.matmul(
                                sc[:, i, :],
                                lhsT=kT[half:half + 64, hm, t4s],
                                rhs=qT[half:half + 64, hm, :],
                                start=True, stop=True,
                            )
                        pt = pt_pool.tile([128, 2, S], BF16)
                        nc.scalar.activation(
                            out=pt.rearrange("p a b -> p (a b)"),
                            in_=sc.rearrange("p a b -> p (a b)"),
                            func=AF.Exp, scale=0.125)
                        pts.append(pt)
                    pt_tiles[h] = pts
                    s01 = psum_pool.tile([128, S], BF16, tag="s16")
                    nc.vector.tensor_add(out=s01, in0=pts[0][:, 0, :],
                                         in1=pts[0][:, 1, :])
                    s23 = psum_pool.tile([128, S], BF16, tag="s16")
                    nc.vector.tensor_add(out=s23, in0=pts[1][:, 0, :],
                                         in1=pts[1][:, 1, :])
                    sum_tiles[h] = (s01, s23)

                def emit_r(h):
                    """r + 1/r + broadcast, emitted one head ahead of AV so
                    the DMA bounce latency is covered by head h-1's AV."""
                    rp = pps.tile([128, S], F32, tag="ps")
                    ns = len(sum_tiles[h])
                    for j, s in enumerate(sum_tiles[h]):
                        nc.tensor.matmul(
                            rp[0:1, :], lhsT=ones_col, rhs=s,
                            start=(j == 0), stop=(j == ns - 1),
                        )
                    rinv = rbc_pool.tile([1, S], F32, tag="rinv")
                    nc.vector.reciprocal_approx_fast(out=rinv, in_=rp[0:1, :])
                    slot = rscr_d.ap()[b * H + h]
                    dout = nc.sync.dma_start(out=slot, in_=rinv)
                    bc = rbc_pool.tile([128, S], F32, tag="bc")
                    f = slot.flatten()
                    din = nc.sync.dma_start(
                        out=bc,
                        in_=bass.AP(tensor=f.tensor, offset=f.offset,
                                    ap=[[0, 128]] + [list(p) for p in f.ap]),
                    )
                    tile.add_dep_helper(din.ins, dout.ins, sync=True,
                                        reason="rinv bounce RAW")
                    bc_tiles[h] = bc
                    del sum_tiles[h]

                av_tiles = {}

                def emit_av(h):
                    hm, half = h // 2, 64 * (h % 2)
                    # head pairs share one av PSUM tile (disjoint partition
                    # halves) so evictions have twice the lookahead room
                    if h % 2 == 0:
                        av = av_ps.tile([128, S], F32, tag="av", name="av")
                        av_tiles[hm] = av
                    av = av_tiles[hm]
                    for t4 in range(4):
                        nc.tensor.matmul(
                            av[half:half + 64, :],
                            lhsT=v_sb[:, t4, h * 64:(h + 1) * 64],
                            rhs=pt_tiles[h][t4 // 2][:, t4 % 2, :],
                            start=(t4 == 0), stop=(t4 == 3),
                        )
                    # fused normalize + PSUM->SBUF eviction
                    nc.vector.tensor_mul(
                        out=oT[half:half + 64, hm, :],
                        in0=av[half:half + 64, :],
                        in1=bc_tiles[h][half:half + 64, :],
                    )
                    del pt_tiles[h], bc_tiles[h]

                emit_scores(0)
                emit_scores(1)
                # batch start: no AV yet to cover the first r chain — pop a
                # couple of interleave units so the PE isn't waiting on exp
                for _ in range(2):
                    if interleave:
                        interleave.pop(0)()
                emit_r(0)
                for h in range(H):
                    if h + 2 < H:
                        emit_scores(h + 2)
                    if h + 1 < H:
                        emit_r(h + 1)
                    emit_av(h)
                    for _ in range(2):
                        if interleave:
                            interleave.pop(0)()

            def p3_units(b):
                """Out-projection as 4 per-token-block units, interleaved
                into the NEXT batch's attention for PE cover."""
                def t4_unit(t4):
                    def emit():
                        oT = ot_tiles[b]
                        ostage = out_pool.tile([128, D], F32)
                        for n in range(2):
                            ps = pps.tile([128, S], F32, tag="ps")
                            for m in range(CD):
                                nc.tensor.matmul(
                                    ps[:, 0:384],
                                    lhsT=oT[:, m, t4 * 128:(t4 + 1) * 128],
                                    rhs=wo_sb[:, m, n * 384:(n + 1) * 384],
                                    start=(m == 0),
                                    stop=(m == CD - 1),
                                )
                            nc.vector.tensor_add(
                                out=ostage[:, n * 384:(n + 1) * 384],
                                in0=ps[:, 0:384],
                                in1=bo_bc[:, n * 384:(n + 1) * 384],
                            )
                        nc.sync.dma_start(
                            out=out_d.ap()[b, t4 * 128:(t4 + 1) * 128, :],
                            in_=ostage,
                        )
                    return emit
                return [t4_unit(t4) for t4 in range(4)]

            # ---- pipeline ----
            for unit in p1_units(0):
                unit()
            prev_p3 = []
            for b in range(BL):
                if b + 1 < BL:
                    emit_x_load(b + 1)
                    units = prev_p3 + p1_units(b + 1)
                else:
                    units = prev_p3
                emit_attention(b, units)
                for unit in units:
                    unit()
                prev_p3 = p3_units(b)
            for unit in prev_p3:
                unit()

    nc.finalize()
    return nc


_NC_CACHE = None


def _get_nc():
    global _NC_CACHE
    if _NC_CACHE is None:
        _NC_CACHE = build_nc()
    return _NC_CACHE


def run_spmd(inputs, trace=False, trace_cores=None):
    nc = _get_nc()
    x = np.ascontiguousarray(inputs["x"], dtype=np.float32)
    shared = {
        k: np.ascontiguousarray(inputs[k], dtype=np.float32)
        for k in ("Wq", "Wk", "Wv", "bq", "bk", "bv", "Wo", "bo")
    }
    in_maps = []
    for core in range(NCORES):
        m = dict(shared)
        m["x"] = np.ascontiguousarray(x[core * BL:(core + 1) * BL])
        in_maps.append(m)
    res = bass_utils.run_bass_kernel_spmd(
        nc, in_maps, core_ids=list(range(NCORES)),
        trace=trace, trace_cores=trace_cores,
    )
    return res


def kernel(**inputs) -> np.ndarray:
    res = run_spmd(inputs, trace=False)
    out = np.concatenate([res.results[i]["out"] for i in range(NCORES)], axis=0)
    return out.astype(np.float32)
